# revision 1
# baseline (speedup 1.0000x reference)
"""CMC loss kernel for Trainium2, sharded across 8 NeuronCores.

Sharding: core i owns view d=i for the negative term (full BxB Gram of
zn[:, i, :]), and the 256-sample slice [256*i, 256*(i+1)) for the positive
term (all 28 view pairs).  Host combines per-core partial sums and does the
final (tiny) logits/logsumexp reduction.
"""

import os
import sys

import numpy as np

sys.path.insert(0, "/opt/trn_rl_repo")

import concourse.bass as bass  # noqa: E402
import concourse.mybir as mybir  # noqa: E402
from concourse.bass_utils import run_bass_kernel_spmd  # noqa: E402
from concourse.library_overlay import lower_extended_insts  # noqa: E402
from concourse.tile import TileContext  # noqa: E402


def _split_waits(nc, max_waits=1):
    """Hoist excess semaphore waits onto standalone event-sem instructions.

    Most TRN2 ISA structs only have sync slots for one wait (plus updates);
    walrus fails with "Too many sync wait commands" otherwise.  An engine
    stalls identically whether a wait rides on the instruction or on an
    InstEventSemaphore immediately before it in the same engine's stream,
    so splitting is semantics-preserving.
    """
    n = 0
    for fn in nc.m.functions:
        for bb in fn.blocks:
            out = []
            changed = False
            for inst in bb.instructions:
                si = inst.sync_info
                if si is not None and si.on_wait and len(si.on_wait) > max_waits:
                    waits = list(si.on_wait)
                    for w in waits[:-max_waits]:
                        out.append(
                            mybir.InstEventSemaphore(
                                name=f"WSPLIT-{n}",
                                engine=inst.engine,
                                ins=[],
                                outs=[],
                                sync_info=mybir.SyncInfo(
                                    on_wait=[w], on_update=[]
                                ),
                            )
                        )
                        n += 1
                    inst.sync_info = mybir.SyncInfo(
                        on_wait=waits[-max_waits:], on_update=si.on_update
                    )
                    changed = True
                out.append(inst)
            if changed:
                bb.instructions = out

B, D, F = 2048, 8, 256
NCORES = 8
BS = B // NCORES  # 256 samples per core (pos term)
P = 128
NB = B // P  # 16 row/b tiles
NF = F // P  # 2 feature halves
CC = 512  # matmul free-dim chunk
NPAIR = D * (D - 1) // 2  # 28 unordered view pairs
NJ = BS // P  # 2 sample tiles for pos term
TEMP = 0.5
INV_TEMP = 1.0 / TEMP

# packed input layout (columns of zin, all f32):
NZV = NB * F  # 4096: z[:, i, :] as [P, NB, F]
NZS = NJ * D * F  # 4096: z[i*BS:(i+1)*BS] as [P, NJ, D, F]
ZIN_W = NZV + NZS

f32 = mybir.dt.float32
bf16 = mybir.dt.bfloat16
ALU = mybir.AluOpType
ACT = mybir.ActivationFunctionType

_CACHED_NC = None


def _build_nc():
    nc = bass.Bass()

    zin = nc.dram_tensor("zin", [P, ZIN_W], f32, kind="ExternalInput")
    # columns 0..NB-1: per-view gram exp row sums (incl. diagonal)
    # columns NB..NB+NJ-1: pos-term pair-exp sums (d<e only)
    out = nc.dram_tensor("out", [P, NB + NJ], f32, kind="ExternalOutput")

    with TileContext(nc) as tc:
        with (
            tc.tile_pool(name="singles", bufs=1) as singles,
            tc.tile_pool(name="work", bufs=3) as work,
            tc.tile_pool(name="small", bufs=4) as small,
            tc.tile_pool(name="psumB", bufs=2, space="PSUM") as psumB,
        ):
            zin_sb = singles.tile([P, ZIN_W], f32)
            nc.sync.dma_start(out=zin_sb, in_=zin[:, :])

            zv_all = zin_sb[:, :NZV].rearrange("p (t f) -> p t f", t=NB)
            zs_all = zin_sb[:, NZV : NZV + NZS].rearrange(
                "p (j d f) -> p j d f", j=NJ, d=D
            )

            # --- Stage A: normalize view slice, transpose to [F, B] ---
            # all 32 per-vector norms (view slice + sample slice) in one
            # big ACT Square + one segmented DVE reduce
            NSEG = NB + NJ * D  # 32 segments of F elements
            sq = singles.tile([P, NSEG * F], f32)
            # square on GPSIMD (near-idle) instead of ScalarE (bottleneck)
            nc.gpsimd.tensor_mul(
                sq, zin_sb[:, : NSEG * F], zin_sb[:, : NSEG * F]
            )
            n2 = singles.tile([P, NSEG], f32)
            nc.vector.tensor_reduce(
                out=n2,
                in_=sq[:, :].rearrange("p (s f) -> p s f", s=NSEG),
                axis=mybir.AxisListType.X,
                op=ALU.add,
            )
            # inv 1/sqrt(n2) = exp(-0.5 * ln(n2)); keeps ACT in one table set
            lnt = small.tile([P, NSEG], f32)
            nc.scalar.activation(lnt, n2, ACT.Ln)
            inv = singles.tile([P, NSEG], f32)
            nc.scalar.activation(inv, lnt, ACT.Exp, scale=-0.5)
            inva = inv[:, :NB]
            invc = inv[:, NB:]

            znT = singles.tile([P, NF, B], bf16)
            for t in range(NB):
                znb = work.tile([P, F], bf16, tag="znb")
                nc.vector.tensor_scalar_mul(
                    znb, zv_all[:, t, :], inva[:, t : t + 1]
                )
                for h in range(NF):
                    # xbar transpose straight into the [F, B] layout;
                    # alternate HWDGE engines for queue parallelism
                    eng = nc.sync if (t + h) % 2 == 0 else nc.scalar
                    eng.dma_start_transpose(
                        znT[:, h, t * P : (t + 1) * P],
                        znb[:, h * P : (h + 1) * P],
                    )

            # --- Stage C: pos term for this core's sample slice ---
            znf = singles.tile([P, NJ, D, F], bf16)
            for j in range(NJ):
                for d in range(D):
                    nc.vector.tensor_scalar_mul(
                        znf[:, j, d, :],
                        zs_all[:, j, d, :],
                        invc[:, j * D + d : j * D + d + 1],
                    )

            dots = singles.tile([P, NJ, NPAIR], f32)
            for j in range(NJ):
                # pair products on the otherwise-idle GPSIMD engine; all 28
                # land in one tile so DVE reduces them in a single op
                prodj = work.tile([P, NPAIR, F], bf16, tag="prodj")
                k = 0
                for d in range(D):
                    for e in range(d + 1, D):
                        nc.gpsimd.tensor_mul(
                            prodj[:, k, :], znf[:, j, d, :], znf[:, j, e, :]
                        )
                        k += 1
                nc.vector.tensor_reduce(
                    out=dots[:, j, :],
                    in_=prodj,
                    axis=mybir.AxisListType.X,
                    op=ALU.add,
                )
            edots = small.tile([P, NJ, NPAIR], f32)
            nc.scalar.activation(edots, dots, ACT.Exp, scale=INV_TEMP)
            possum = singles.tile([P, NJ], f32)
            for j in range(NJ):
                nc.vector.tensor_reduce(
                    out=possum[:, j : j + 1],
                    in_=edots[:, j, :],
                    axis=mybir.AxisListType.X,
                    op=ALU.add,
                )

            # --- Stage B: full BxB gram for this core's view ---
            rowsums = singles.tile([P, NB], f32)
            for rb in range(NB):
                ps = psumB.tile([P, B], f32, tag="ps")
                for cc in range(B // CC):
                    for h in range(NF):
                        nc.tensor.matmul(
                            ps[:, cc * CC : (cc + 1) * CC],
                            znT[:, h, rb * P : (rb + 1) * P],
                            znT[:, h, cc * CC : (cc + 1) * CC],
                            start=(h == 0),
                            stop=(h == NF - 1),
                        )
                ejunk = work.tile([P, B], bf16, tag="ejunk")
                nc.scalar.activation(
                    ejunk, ps, ACT.Exp, scale=INV_TEMP,
                    accum_out=rowsums[:, rb : rb + 1],
                )

            # --- outputs ---
            outsb = singles.tile([P, NB + NJ], f32)
            nc.vector.tensor_copy(outsb[:, :NB], rowsums)
            nc.vector.tensor_copy(outsb[:, NB:], possum)
            nc.sync.dma_start(out=out[:, :], in_=outsb)

    if os.environ.get("KERNEL_NO_SPLIT") != "1":  # CoreSim can't run the
        _split_waits(nc)  # post-hoc event-sem instructions; HW needs them
    lower_extended_insts(nc)
    return nc


def _get_nc():
    global _CACHED_NC
    if _CACHED_NC is None:
        _CACHED_NC = _build_nc()
    return _CACHED_NC


def _pack_core_input(z, i):
    zv = z[:, i, :].reshape(NB, P, F).transpose(1, 0, 2).reshape(P, NZV)
    zs = (
        z[i * BS : (i + 1) * BS]
        .reshape(NJ, P, D, F)
        .transpose(1, 0, 2, 3)
        .reshape(P, NZS)
    )
    return np.ascontiguousarray(np.concatenate([zv, zs], axis=1))


def _run(z, trace=False):
    z = np.ascontiguousarray(np.asarray(z, dtype=np.float32))
    assert z.shape == (B, D, F), z.shape
    in_maps = [{"zin": _pack_core_input(z, i)} for i in range(NCORES)]
    nc = _get_nc()
    res = run_bass_kernel_spmd(
        nc, in_maps, core_ids=list(range(NCORES)), trace=trace
    )
    return res


def _finish(results):
    neg_raw = np.zeros(B, np.float64)
    pos_half = np.zeros(B, np.float64)
    for i, r in enumerate(results):
        o = np.asarray(r["out"], np.float64)  # [P, NB + NJ]
        rowsums = o[:, :NB]  # [P, NB] ; sample = t*128 + p
        possums = o[:, NB:]  # [P, NJ] ; sample = i*BS + j*128 + p
        neg_raw += rowsums.T.reshape(B)
        pos_half[i * BS : (i + 1) * BS] = possums.T.reshape(BS)

    e2 = np.exp(INV_TEMP)  # exp(1/T * 1.0) diagonal term
    neg = (neg_raw - D * e2) / (B - 1)
    pos = 2.0 * pos_half
    logits = pos / (pos + neg)
    m = logits.max()
    lse = np.log(np.sum(np.exp(logits - m))) + m
    loss = lse - logits.mean()
    return np.float32(loss)


def kernel(**inputs) -> np.ndarray:
    res = _run(inputs["z"], trace=False)
    return _finish(res.results)



# revision 5
# speedup vs baseline: 1.5867x; 1.5867x over previous
"""CMC loss kernel for Trainium2, sharded across 8 NeuronCores.

Sharding: core i owns view d=i for the negative term (full BxB Gram of
zn[:, i, :]), and the 256-sample slice [256*i, 256*(i+1)) for the positive
term (all 28 view pairs).  Host combines per-core partial sums and does the
final (tiny) logits/logsumexp reduction.

v2 layout: the host pre-transposes the view slice to [F, B] (bf16), so the
kernel needs no on-chip transposes.  Per-sample norms come from a GpSimd
partition all-reduce of the squared columns; the inverse norms (x64 fp8
prescale) are applied column-wise, the scaled matrix is cast to fp8e4, and
the Gram runs as DoubleRow fp8 matmuls (full F=256 contraction per
instruction).  The positive term works on a sample-major slice with
offset-sliced pair products (7 big DVE ops) on raw data, scaled by the
pairwise inverse-norm products afterwards.
"""

import math
import os
import sys

import numpy as np

sys.path.insert(0, "/opt/trn_rl_repo")

import concourse.bass as bass  # noqa: E402
import concourse.bass_isa as bass_isa  # noqa: E402
import concourse.mybir as mybir  # noqa: E402
from concourse.bass_utils import run_bass_kernel_spmd  # noqa: E402
from concourse.library_overlay import lower_extended_insts  # noqa: E402
from concourse.tile import TileContext  # noqa: E402

import ml_dtypes  # noqa: E402


def _split_waits(nc, max_waits=1):
    """Hoist excess semaphore waits onto standalone event-sem instructions.

    Most TRN2 ISA structs only have sync slots for one wait (plus updates);
    walrus fails with "Too many sync wait commands" otherwise.  An engine
    stalls identically whether a wait rides on the instruction or on an
    InstEventSemaphore immediately before it in the same engine's stream,
    so splitting is semantics-preserving.
    """
    n = 0
    for fn in nc.m.functions:
        for bb in fn.blocks:
            out = []
            changed = False
            for inst in bb.instructions:
                si = inst.sync_info
                if si is not None and si.on_wait and len(si.on_wait) > max_waits:
                    waits = list(si.on_wait)
                    for w in waits[:-max_waits]:
                        out.append(
                            mybir.InstEventSemaphore(
                                name=f"WSPLIT-{n}",
                                engine=inst.engine,
                                ins=[],
                                outs=[],
                                sync_info=mybir.SyncInfo(
                                    on_wait=[w], on_update=[]
                                ),
                            )
                        )
                        n += 1
                    inst.sync_info = mybir.SyncInfo(
                        on_wait=waits[-max_waits:], on_update=si.on_update
                    )
                    changed = True
                out.append(inst)
            if changed:
                bb.instructions = out


B, D, F = 2048, 8, 256
NCORES = 8
BS = B // NCORES  # 256 samples per core (pos term)
P = 128
NB = B // P  # 16 row blocks of the gram
NH = F // P  # 2 feature halves
CC = 512  # matmul free-dim chunk (one PSUM bank)
NJ = BS // P  # 2 sample tiles for pos term
NPAIR = D * (D - 1) // 2  # 28 unordered view pairs
TEMP = 0.5
INV_TEMP = 1.0 / TEMP
FS = 64.0  # fp8 prescale folded into the inverse norms

# packed input layout (bf16 columns of zin):
ZVT_W = NH * B  # 4096: z[:, i, :].T as [P, NH, B]  (f-major)
ZS_W = NJ * D * F  # 4096: z[i*BS:(i+1)*BS] as [P, NJ, D, F] (sample-major)
ZIN_W = ZVT_W + ZS_W

f32 = mybir.dt.float32
bf16 = mybir.dt.bfloat16
fp8 = mybir.dt.float8e4
ALU = mybir.AluOpType
ACT = mybir.ActivationFunctionType

_CACHED_NC = None


def _build_nc():
    nc = bass.Bass()

    zin = nc.dram_tensor("zin", [P, ZIN_W], bf16, kind="ExternalInput")
    # columns 0..NB-1: per-view gram exp row sums (incl. ~e^2 diagonal)
    # columns NB..NB+NJ-1: pos-term pair-exp sums (d<e only)
    out = nc.dram_tensor("out", [P, NB + NJ], f32, kind="ExternalOutput")

    with TileContext(nc) as tc:
        with (
            tc.tile_pool(name="singles", bufs=1) as singles,
            tc.tile_pool(name="work", bufs=3) as work,
            tc.tile_pool(name="small", bufs=4) as small,
            tc.tile_pool(name="psumB", bufs=2, space="PSUM") as psumB,
        ):
            zin_sb = singles.tile([P, ZIN_W], bf16)
            # chunked input load: view-slice halves first (gram critical
            # path), pos slice on a second queue
            nc.sync.dma_start(out=zin_sb[:, :2048], in_=zin[:, :2048])
            nc.sync.dma_start(
                out=zin_sb[:, 2048:ZVT_W], in_=zin[:, 2048:ZVT_W]
            )
            nc.gpsimd.dma_start(out=zin_sb[:, ZVT_W:], in_=zin[:, ZVT_W:])

            zvt = zin_sb[:, :ZVT_W].rearrange("p (h b) -> p h b", h=NH)
            zs = zin_sb[:, ZVT_W:].rearrange(
                "p (j d f) -> p j d f", j=NJ, d=D
            )
            zsf = zin_sb[:, ZVT_W:].rearrange("p (s f) -> p s f", s=NJ * D)

            # --- view norms: colsum of squares via partition all-reduce ---
            sq = work.tile([P, NH, B], bf16, tag="sq")
            nc.vector.tensor_mul(sq[:, 0, :], zvt[:, 0, :], zvt[:, 0, :])
            nc.vector.tensor_mul(sq[:, 1, :], zvt[:, 1, :], zvt[:, 1, :])
            sqh = work.tile([P, B], bf16, tag="sqh")
            nc.vector.tensor_add(sqh, sq[:, 0, :], sq[:, 1, :])
            n2b = singles.tile([P, B], f32)
            nc.gpsimd.partition_all_reduce(
                n2b[:, :], sqh[:, :], channels=P,
                reduce_op=bass_isa.ReduceOp.add,
            )
            # invb = FS / sqrt(n2) = exp(-0.5 ln(n2 / FS^2)), per column
            lnb = work.tile([P, B], f32, tag="lnb")
            nc.scalar.activation(lnb, n2b, ACT.Ln, scale=1.0 / (FS * FS))
            invb = singles.tile([P, B], bf16)
            nc.scalar.activation(invb, lnb, ACT.Exp, scale=-0.5)

            # --- pos-slice norms (free-axis reduce) ---
            sqs = work.tile([P, NJ * D, F], bf16, tag="sqs")
            nc.vector.tensor_mul(sqs, zsf, zsf)
            n2s = small.tile([P, NJ * D], bf16)
            with nc.allow_low_precision(
                reason="n2 ~256, bf16 rounding is 0.4%; final tol 2e-2"
            ):
                nc.vector.tensor_reduce(
                    out=n2s, in_=sqs, axis=mybir.AxisListType.X, op=ALU.add
                )
            lns = small.tile([P, NJ * D], f32)
            nc.scalar.activation(lns, n2s, ACT.Ln)
            invs = small.tile([P, NJ * D], f32)
            nc.scalar.activation(invs, lns, ACT.Exp, scale=-0.5)

            # --- scaled fp8 copy of the transposed view slice ---
            zts = singles.tile([P, NH, B], fp8)
            nc.vector.tensor_mul(zts[:, 0, :], zvt[:, 0, :], invb)
            nc.vector.tensor_mul(zts[:, 1, :], zvt[:, 1, :], invb)

            # --- pos pair products on RAW data (offset-sliced) ---
            prod = singles.tile([P, NJ, NPAIR, F], bf16)
            ofs = 0
            for o in range(1, D):
                w = D - o
                nc.vector.tensor_mul(
                    prod[:, :, ofs : ofs + w, :],
                    zs[:, :, 0:w, :],
                    zs[:, :, o:D, :],
                )
                ofs += w
            rawdot = small.tile([P, NJ, NPAIR], bf16)
            with nc.allow_low_precision(
                reason="pair dots |.|<40 rounded to bf16; final tol 2e-2"
            ):
                nc.vector.tensor_reduce(
                    out=rawdot, in_=prod, axis=mybir.AxisListType.X,
                    op=ALU.add,
                )
            invv = invs.rearrange("p (j d) -> p j d", j=NJ)
            invprod = small.tile([P, NJ, NPAIR], f32)
            ofs = 0
            for o in range(1, D):
                w = D - o
                nc.vector.tensor_mul(
                    invprod[:, :, ofs : ofs + w],
                    invv[:, :, 0:w],
                    invv[:, :, o:D],
                )
                ofs += w
            sdots = small.tile([P, NJ, NPAIR], f32)
            nc.vector.tensor_mul(sdots, rawdot, invprod)

            # --- gram: fp8 DoubleRow matmuls, full F contraction each ---
            rowsums = singles.tile([P, NB], f32)
            for rb in range(NB):
                ps = psumB.tile([P, B], f32, tag="ps")
                for c in range(B // CC):
                    nc.tensor.matmul(
                        ps[:, c * CC : (c + 1) * CC],
                        zts[:, :, rb * P : (rb + 1) * P],
                        zts[:, :, c * CC : (c + 1) * CC],
                        start=True,
                        stop=True,
                        perf_mode=mybir.MatmulPerfMode.DoubleRow,
                    )
                ejunk = work.tile([P, B], bf16, tag="ejunk")
                nc.scalar.activation(
                    ejunk, ps, ACT.Exp, scale=INV_TEMP / (FS * FS),
                    accum_out=rowsums[:, rb : rb + 1],
                )

            # --- pos exp + accumulate (after gram exps in queue order) ---
            possum = singles.tile([P, NJ], f32)
            pjunk = small.tile([P, NJ, NPAIR], bf16)
            for j in range(NJ):
                nc.scalar.activation(
                    pjunk[:, j, :], sdots[:, j, :], ACT.Exp,
                    scale=INV_TEMP, accum_out=possum[:, j : j + 1],
                )

            # --- outputs ---
            outsb = singles.tile([P, NB + NJ], f32)
            nc.vector.tensor_copy(outsb[:, :NB], rowsums)
            nc.vector.tensor_copy(outsb[:, NB:], possum)
            nc.sync.dma_start(out=out[:, :], in_=outsb)

    _insert_library_loads(nc)
    if os.environ.get("KERNEL_NO_SPLIT") != "1":  # CoreSim can't run the
        _split_waits(nc)  # post-hoc event-sem instructions; HW needs them
    lower_extended_insts(nc)
    return nc


def _insert_library_loads(nc):
    """GpSimd library loads for partition_all_reduce (attn library).

    Same pass Bacc.compile runs; raw Bass skips it, but the Pool
    all-reduce is an extended inst that needs its ucode library resident.
    """
    import bass_rust as _bass_rust
    from concourse.library_config import all_libraries, standard

    inst_type_to_lib_mask = {}
    for lib in all_libraries:
        for inst_type in lib.instructions:
            inst_type_to_lib_mask[inst_type] = inst_type_to_lib_mask.get(
                inst_type, 0
            ) | (1 << lib.index)
    _bass_rust.insert_library_loads(
        nc, inst_type_to_lib_mask, len(all_libraries), standard.index
    )


def _get_nc():
    global _CACHED_NC
    if _CACHED_NC is None:
        _CACHED_NC = _build_nc()
    return _CACHED_NC


def _pack_core_input(z, i):
    # view slice, f-major: zvT[p, h, b] = z[b, i, 128h+p]
    zvt = (
        z[:, i, :]
        .T.reshape(NH, P, B)
        .transpose(1, 0, 2)
        .reshape(P, ZVT_W)
    )
    # pos slice, sample-major: zs[p, j, d, f] = z[i*BS + j*128 + p, d, f]
    zsl = (
        z[i * BS : (i + 1) * BS]
        .reshape(NJ, P, D, F)
        .transpose(1, 0, 2, 3)
        .reshape(P, ZS_W)
    )
    packed = np.concatenate([zvt, zsl], axis=1)
    return np.ascontiguousarray(packed.astype(ml_dtypes.bfloat16))


def _run(z, trace=False):
    z = np.ascontiguousarray(np.asarray(z, dtype=np.float32))
    assert z.shape == (B, D, F), z.shape
    in_maps = [{"zin": _pack_core_input(z, i)} for i in range(NCORES)]
    nc = _get_nc()
    res = run_bass_kernel_spmd(
        nc, in_maps, core_ids=list(range(NCORES)), trace=trace
    )
    return res


def _finish(results):
    neg_raw = np.zeros(B, np.float64)
    pos_half = np.zeros(B, np.float64)
    for i, r in enumerate(results):
        o = np.asarray(r["out"], np.float64)  # [P, NB + NJ]
        rowsums = o[:, :NB]  # [P, NB] ; sample = t*128 + p
        possums = o[:, NB:]  # [P, NJ] ; sample = i*BS + j*128 + p
        neg_raw += rowsums.T.reshape(B)
        pos_half[i * BS : (i + 1) * BS] = possums.T.reshape(BS)

    e2 = np.exp(INV_TEMP)  # exp(1/T * 1.0) diagonal term
    neg = (neg_raw - D * e2) / (B - 1)
    pos = 2.0 * pos_half
    logits = pos / (pos + neg)
    m = logits.max()
    lse = np.log(np.sum(np.exp(logits - m))) + m
    loss = lse - logits.mean()
    return np.float32(loss)


def kernel(**inputs) -> np.ndarray:
    res = _run(inputs["z"], trace=False)
    return _finish(res.results)
